# revision 30
# baseline (speedup 1.0000x reference)
"""Trainium2 Bass kernel for the 3-layer MLP encode/decode forward pass.

Computation (B = 65536):
    d_i = pinv(W_i)                       (host, negligible)
    h = lrelu(x @ W1.T)                   [B, 128]
    h = lrelu(h @ W2.T)                   [B, 64]
    h = h @ W3.T                          [B, 16]
    h = lrelu(h @ d3.T)                   [B, 64]   (folded: lrelu((d3@W3) @ h2))
    h = lrelu(h @ d2.T)                   [B, 128]
    out = h @ d1.T                        [B, 784]

Sharding: pure data-parallel — 8 cores x 8192 batch rows each; the tiny
weights (and host-side pinv) are replicated.

Layout: x is pre-transposed to feature-major fp16 on the host (free, like
the pinv) and stored group-major so each input DMA is one contiguous
28KB-per-partition block.  The batch order inside each 512-column tile is
permuted so the final layer's batch-major PSUM output lands in natural
row order.  All matmul operands are fp16 (rel-err ~6e-4 vs the 2e-2
gate); fp16 in/out halves HBM traffic vs fp32: 12.8 MB in + 12.8 MB out
per core -> ~72 us roofline at ~360 GB/s per-core HBM bandwidth.

Schedule: software-pipelined across layers with one 512-column tile of
skew per layer — step t issues L1(t), L2(t-1), L3(t-2), L4(t-3),
L5(t-4)+store(t-4).  Every consumer reads data produced a full step
(~4us) earlier, so the PE never waits on the ACT lrelu round trip, ACT
does only lrelus, and the final-layer PSUM->SBUF copies are split in
half-tiles drained concurrently by DVE and GpSimd.
"""

import numpy as np

B = 65536
N_CORES = 8
B_LOC = B // N_CORES  # 8192
D0, D1, D2, D3 = 784, 128, 64, 16
KCH = 112          # 784 = 7 * 112 contraction chunks for layer 1
NKC = D0 // KCH    # 7
TILE = 512         # moving free dim per matmul (one fp32 PSUM bank)
SUB = 128          # batch sub-tile (partition dim of out tiles)
NSUB = TILE // SUB  # 4
HALF = D0 // 2     # 392
IN_W = 4           # tiles per input DMA group


def _build_nc(b_loc=B_LOC, repeat=1, prefetch=6, xin_bufs=7, outp_bufs=4,
              acts_bufs=3, ps_mm_bufs=4, ps_o_bufs=2,
              in_dma_eng="sync", out_dma_eng="scalar",
              o_copy_eng=("vector", "vector", "vector", "vector"),
              o_copy_eng_b=("scalar", "scalar", "vector", "vector"),
              alloc_mode="stack", staggered=False, unroll=1, ablate=""):
    import contextlib
    import concourse.tile as tile
    from concourse import bacc, mybir

    f32 = mybir.dt.float32
    f16 = mybir.dt.float16
    LRELU = mybir.ActivationFunctionType.Lrelu
    COPY = mybir.ActivationFunctionType.Copy

    nc = bacc.Bacc(trn_type="TRN2", target_bir_lowering=False, debug=False,
                   num_devices=N_CORES)

    n_tiles = b_loc // TILE

    xt = nc.declare_dram_parameter("xt", [n_tiles * KCH, NKC * TILE], f16,
                                   isOutput=False).ap()
    w1t = nc.declare_dram_parameter("w1t", [D0, D1], f16, isOutput=False).ap()
    w2t = nc.declare_dram_parameter("w2t", [D1, D2], f16, isOutput=False).ap()
    m3t = nc.declare_dram_parameter("m3t", [D2, D2], f16, isOutput=False).ap()
    d2t = nc.declare_dram_parameter("d2t", [D2, D1], f16, isOutput=False).ap()
    d1t = nc.declare_dram_parameter("d1t", [D1, D0], f16, isOutput=False).ap()
    out = nc.declare_dram_parameter("out", [b_loc, D0], f16, isOutput=True).ap()

    xt_r = xt.rearrange("(t p) f -> t p f", p=KCH)
    out_r = out.rearrange("(n p s) f -> n p (s f)", p=SUB, s=NSUB)

    def copy_eng(name):
        if name == "scalar":
            return lambda o, i: nc.scalar.activation(out=o, in_=i, func=COPY)
        return nc.vector.tensor_copy

    with tile.TileContext(nc, num_cores=N_CORES, pool_alloc_mode=alloc_mode) as tc:
        with (
            tc.tile_pool(name="consts", bufs=1) as consts,
            tc.tile_pool(name="xin", bufs=xin_bufs) as xin,
            tc.tile_pool(name="acts", bufs=acts_bufs) as acts,
            tc.tile_pool(name="outp", bufs=outp_bufs) as outp,
            tc.tile_pool(name="psMM", bufs=ps_mm_bufs, space="PSUM") as psMM,
            tc.tile_pool(name="psA", bufs=ps_o_bufs, space="PSUM") as psA,
            tc.tile_pool(name="psB", bufs=ps_o_bufs, space="PSUM") as psB,
        ):
            # --- constants (loaded once, outside the timing loop) ---
            w1t_sb = consts.tile([KCH, NKC, D1], f16)
            nc.sync.dma_start(out=w1t_sb, in_=w1t.rearrange("(c p) m -> p c m", p=KCH))
            w2t_sb = consts.tile([D1, D2], f16)
            nc.sync.dma_start(out=w2t_sb, in_=w2t)
            m3t_sb = consts.tile([D2, D2], f16)
            nc.sync.dma_start(out=m3t_sb, in_=m3t)
            d2t_sb = consts.tile([D2, D1], f16)
            nc.sync.dma_start(out=d2t_sb, in_=d2t)
            d1t_sb = consts.tile([D1, D0], f16)
            nc.sync.dma_start(out=d1t_sb, in_=d1t)

            # dummy SBUF sources for ablation builds (timing-only)
            if ablate == "dma_pe":
                dum1 = consts.tile([D1, TILE], f16)
                nc.sync.dma_start(out=dum1, in_=d1t[:, :TILE])

            assert repeat % unroll == 0
            rep_ctx = (tc.For_i(0, repeat // unroll, 1,
                                staggered_reset=bool(staggered))
                       if repeat > unroll else contextlib.nullcontext())
            with rep_ctx:
                x_sb = {}   # tile -> sbuf tile
                h1, h2, g3, g2 = {}, {}, {}, {}
                n_unr = n_tiles * unroll

                def in_eng(i):
                    if in_dma_eng == "alt":
                        return nc.sync if i % 2 == 0 else nc.scalar
                    if in_dma_eng == "rr3":
                        return (nc.sync, nc.scalar, nc.gpsimd)[i % 3]
                    return getattr(nc, in_dma_eng)

                def out_eng(i):
                    if out_dma_eng == "alt":
                        return nc.scalar if i % 2 == 0 else nc.sync
                    if out_dma_eng == "rr3":
                        return (nc.gpsimd, nc.scalar, nc.sync)[i % 3]
                    return getattr(nc, out_dma_eng)

                def load_tile(i):
                    if i < n_unr and ablate != "dma_out":
                        x_sb[i] = xin.tile([KCH, NKC * TILE], f16, tag="x",
                                           name="x_sb")
                        in_eng(i).dma_start(out=x_sb[i], in_=xt_r[i % n_tiles])

                if ablate:  # timing-only ablation: DMA (+ PE) skeleton
                    for i in range(prefetch):
                        load_tile(i)
                    for step in range(n_unr + 4):
                        load_tile(step + prefetch)
                        t = step
                        if t < n_unr and ablate != "dma_out":
                            xt_t = x_sb.pop(t)
                            if ablate == "dma_pe":
                                h1_ps = psMM.tile([D1, TILE], f32, tag="mm")
                                for c in range(NKC):
                                    nc.tensor.matmul(
                                        h1_ps, lhsT=w1t_sb[:, c, :],
                                        rhs=xt_t[:, c * TILE:(c + 1) * TILE],
                                        start=(c == 0), stop=(c == NKC - 1))
                                h2_ps = psMM.tile([D2, TILE], f32, tag="mm")
                                nc.tensor.matmul(h2_ps, lhsT=w2t_sb,
                                                 rhs=dum1, start=True,
                                                 stop=True)
                                g3_ps = psMM.tile([D2, TILE], f32, tag="mm")
                                nc.tensor.matmul(g3_ps, lhsT=m3t_sb,
                                                 rhs=dum1[:D2, :], start=True,
                                                 stop=True)
                                g2_ps = psMM.tile([D1, TILE], f32, tag="mm")
                                nc.tensor.matmul(g2_ps, lhsT=d2t_sb,
                                                 rhs=dum1[:D2, :], start=True,
                                                 stop=True)
                        t = step - 4
                        if 0 <= t < n_unr and ablate != "dma_in":
                            o_sb = outp.tile([SUB, NSUB, D0], f16, tag="o")
                            nc.vector.memset(o_sb[:, 0, :8], 0.0)
                            if ablate == "dma_pe":
                                for s in range(NSUB):
                                    dumc = dum1[:, s * SUB:(s + 1) * SUB]
                                    po_a = psA.tile([SUB, TILE], f32, tag="poa")
                                    po_b = psB.tile([SUB, TILE], f32, tag="pob")
                                    nc.tensor.matmul(po_a[:, :HALF], lhsT=dumc,
                                                     rhs=d1t_sb[:, :HALF],
                                                     start=True, stop=True)
                                    nc.tensor.matmul(po_b[:, :HALF], lhsT=dumc,
                                                     rhs=d1t_sb[:, HALF:],
                                                     start=True, stop=True)
                            out_eng(t).dma_start(
                                out=out_r[t % n_tiles], in_=o_sb)

                for i in range(prefetch) if not ablate else []:
                    load_tile(i)
                for step in range(n_unr + 4) if not ablate else []:
                    load_tile(step + prefetch)

                    t = step
                    if t < n_unr:  # --- L1: h1 = lrelu(W1 @ xT) [128,512]
                        xt_t = x_sb.pop(t)
                        h1_ps = psMM.tile([D1, TILE], f32, tag="mm")
                        for c in range(NKC):
                            nc.tensor.matmul(
                                h1_ps, lhsT=w1t_sb[:, c, :],
                                rhs=xt_t[:, c * TILE:(c + 1) * TILE],
                                start=(c == 0), stop=(c == NKC - 1))
                        h1[t] = acts.tile([D1, TILE], f16, tag="h1",
                                          name="h1_sb")
                        nc.scalar.activation(out=h1[t], in_=h1_ps, func=LRELU,
                                             alpha=0.01)

                    t = step - 1
                    if 0 <= t < n_unr:  # --- L2 [64,512]
                        h2_ps = psMM.tile([D2, TILE], f32, tag="mm")
                        nc.tensor.matmul(h2_ps, lhsT=w2t_sb, rhs=h1.pop(t),
                                         start=True, stop=True)
                        h2[t] = acts.tile([D2, TILE], f16, tag="h2",
                                          name="h2_sb")
                        nc.scalar.activation(out=h2[t], in_=h2_ps, func=LRELU,
                                             alpha=0.01)

                    t = step - 2
                    if 0 <= t < n_unr:  # --- L3 folded (d3@W3) [64,512]
                        g3_ps = psMM.tile([D2, TILE], f32, tag="mm")
                        nc.tensor.matmul(g3_ps, lhsT=m3t_sb, rhs=h2.pop(t),
                                         start=True, stop=True)
                        g3[t] = acts.tile([D2, TILE], f16, tag="g3",
                                          name="g3_sb")
                        nc.scalar.activation(out=g3[t], in_=g3_ps, func=LRELU,
                                             alpha=0.01)

                    t = step - 3
                    if 0 <= t < n_unr:  # --- L4 [128,512]
                        g2_ps = psMM.tile([D1, TILE], f32, tag="mm")
                        nc.tensor.matmul(g2_ps, lhsT=d2t_sb, rhs=g3.pop(t),
                                         start=True, stop=True)
                        g2[t] = acts.tile([D1, TILE], f16, tag="g2",
                                          name="g2_sb")
                        nc.scalar.activation(out=g2[t], in_=g2_ps, func=LRELU,
                                             alpha=0.01)

                    t = step - 4
                    if 0 <= t < n_unr:  # --- L5 + store, batch-major
                        g2t = g2.pop(t) if not ablate else None
                        o_sb = outp.tile([SUB, NSUB, D0], f16, tag="o")
                        for s in range(NSUB) if not ablate else []:
                            g2c = g2t[:, s * SUB:(s + 1) * SUB]
                            po_a = psA.tile([SUB, TILE], f32, tag="poa")
                            po_b = psB.tile([SUB, TILE], f32, tag="pob")
                            nc.tensor.matmul(po_a[:, :HALF], lhsT=g2c,
                                             rhs=d1t_sb[:, :HALF],
                                             start=True, stop=True)
                            nc.tensor.matmul(po_b[:, :HALF], lhsT=g2c,
                                             rhs=d1t_sb[:, HALF:],
                                             start=True, stop=True)
                            copy_eng(o_copy_eng[s])(
                                o_sb[:, s, :HALF], po_a[:, :HALF])
                            copy_eng(o_copy_eng_b[s])(
                                o_sb[:, s, HALF:], po_b[:, :HALF])
                        out_eng(t).dma_start(out=out_r[t % n_tiles], in_=o_sb)

    nc.finalize()
    return nc


def _host_weights(W1, W2, W3):
    def pinv(W):
        u, s, vh = np.linalg.svd(W.astype(np.float64), full_matrices=False)
        return (vh.T * (1.0 / s)) @ u.T

    d1, d2, d3 = pinv(W1), pinv(W2), pinv(W3)
    f = np.float16
    return {
        "w1t": np.ascontiguousarray(np.asarray(W1).T, dtype=f),
        "w2t": np.ascontiguousarray(np.asarray(W2).T, dtype=f),
        "m3t": np.ascontiguousarray((d3 @ np.asarray(W3).astype(np.float64)).T, dtype=f),
        "d2t": np.ascontiguousarray(d2.T, dtype=f),
        "d1t": np.ascontiguousarray(d1.T, dtype=f),
    }


def _prep_x(x):
    """Per-core tile-major feature-major fp16 x.

    Output[i] has shape [16*112, 7*512]; row t*112+p, col c*512+j holds
    x[i*8192 + t*512 + q*4 + s, c*112+p] for j = s*128 + q — i.e.
    feature-major with the within-tile batch permutation that makes the
    device output land in natural row order, one contiguous 7KB block per
    SBUF partition per tile.
    """
    x = np.asarray(x)
    nt = B_LOC // TILE
    outs = []
    for i in range(N_CORES):
        xc = x[i * B_LOC:(i + 1) * B_LOC]
        a = xc.reshape(nt, SUB, NSUB, D0).transpose(0, 2, 1, 3)
        a = a.reshape(B_LOC, D0)          # row b = t*512 + s*128 + q
        xT = a.T.astype(np.float16)       # [784, 8192]
        grp = xT.reshape(NKC, KCH, nt, TILE).transpose(2, 1, 0, 3)
        outs.append(np.ascontiguousarray(grp.reshape(nt * KCH, NKC * TILE)))
    return outs


_NC_CACHE = {}


def _get_nc(key=()):
    if key not in _NC_CACHE:
        _NC_CACHE[key] = _build_nc(B_LOC)
    return _NC_CACHE[key]


def kernel(x, W1, W2, W3):
    from concourse.bass_utils import run_bass_kernel_spmd

    w = _host_weights(np.asarray(W1), np.asarray(W2), np.asarray(W3))
    xts = _prep_x(x)
    nc = _get_nc()
    in_maps = [{"xt": xts[i], **w} for i in range(N_CORES)]
    res = run_bass_kernel_spmd(nc, in_maps, core_ids=list(range(N_CORES)))
    return np.concatenate(
        [res.results[i]["out"].astype(np.float32) for i in range(N_CORES)],
        axis=0)
